# revision 6
# baseline (speedup 1.0000x reference)
"""Trainium2 Bass kernel for nn_BatchSoftmaxNomax (batch contrastive softmax loss).

Math: scores[b,c,n,f] = <ner[b,n,:], face[c,f,:]>, logits = scores.mean((n,f)),
loss = -mean_b log_softmax(logits)[b,b].
Since the span-means are linear, logits[b,c] = <mean_n ner[b], mean_f face[c]>,
so the O(B^2*N^2*D) einsum collapses to two mean-reductions + a [B,D]x[D,B] matmul.

Sharding (8 cores, batch-sharded), two launches with a host-side gather between
them (a device AllGather costs ~55us of cross-rank launch-skew wait through this
runtime — measured — so two independent launches win).

Launch A (per core, 32 batch rows): host packs both input slices as ONE fp8
tensor [128, 8192] laid out d-major/span-minor (p = 4m + n//8, line = [d, j]),
so each of the 4 streaming DMAs is 2KB-contiguous per partition. fp8 halves the
stream vs bf16; the span-mean averages quantization noise (~1e-3 on the loss).
DVE tensor_reduce(axis=X) sums the 8 spans per partition in one op per tile,
a bf16 cast + one [128,32]x[128,512] PE matmul per tensor finishes the
32-row mean. Output: packed [32, 1024] bf16 means (nm | fm).

Host: gathers/transposes the means into fmt [128, 1024] (k-major) and per-core
nmt [128, 128], computes the 256 diagonal dot products in f32.

Launch B (per core): 3 contiguous DMAs, 4 accumulating bf16 matmuls -> [32, 256]
logits in PSUM; ACT exp with fused row-sum accumulate -> sum_c exp(logits).
Host: loss = -mean(diag - log(rowsum)).
"""

import ml_dtypes
import numpy as np
from contextlib import ExitStack

B = 256      # global batch
N1 = 32      # ner spans
N2 = 32      # face spans
D = 512      # embed dim
M = 8        # cores
BL = B // M  # local batch rows per core (32)
KD = D // 128  # d-chunks (4)
PJ = 8       # spans folded into each partition line
PCOLS = D * PJ          # 4096 fp8 bytes per partition per tensor
NTILE = PCOLS // 2      # 2048 — half-tensor DMA tile width

_CACHE = {}


def _emit_a(ctx, tc, means_out, data, sel4):
    from concourse import mybir

    nc = tc.nc
    f32 = mybir.dt.float32
    bf16 = mybir.dt.bfloat16
    fp8 = mybir.dt.float8e4

    consts = ctx.enter_context(tc.tile_pool(name="consts", bufs=1))
    chunks = ctx.enter_context(tc.tile_pool(name="chunks", bufs=1))
    work = ctx.enter_context(tc.tile_pool(name="work", bufs=1))
    mpsum = ctx.enter_context(tc.tile_pool(name="mpsum", bufs=2, space="PSUM"))

    # sel first (tiny, needed by every matmul), then 4 streaming tiles of
    # 256KB (2KB contiguous per partition line; j-major [j, d] lines).
    # Same-tensor halves ride different HWDGE rings.
    sel_sb = consts.tile([128, BL], fp8)
    nc.sync.dma_start(sel_sb[:], sel4)
    tiles = []
    order = [(nc.sync, 0), (nc.scalar, 1), (nc.sync, 2), (nc.scalar, 3)]
    for q, t in order:
        tl = chunks.tile([128, NTILE], fp8, tag=f"t{t}", name=f"t{t}")
        q.dma_start(tl[:], data[:, t * NTILE:(t + 1) * NTILE])
        tiles.append((t, tl))
    tiles.sort()

    # Span-mean on PE: per tensor, 8 accumulating matmuls against sel (1/32
    # selection), moving [128, 512] fp8 slices; chains interleave in data-
    # arrival order (n0, f0, n1, f1) across two PSUM banks.
    ps = [
        mpsum.tile([BL, D], f32, tag=f"ps{i}", name=f"ps{i}")
        for i in range(2)
    ]
    means = work.tile([BL, 2 * D], bf16, tag="means")
    copy_eng = [nc.vector.tensor_copy, nc.scalar.copy]
    for i in range(2):
        for half in (0, 1):
            tl = tiles[2 * i + half][1]
            for j in range(PJ // 2):
                nc.tensor.matmul(
                    ps[i][:], sel_sb[:], tl[:, j * D:(j + 1) * D],
                    start=(half == 0 and j == 0),
                    stop=(half == 1 and j == PJ // 2 - 1),
                )
        copy_eng[i](means[:, i * D:(i + 1) * D], ps[i][:])
    nc.sync.dma_start(means_out, means[:])


def _emit_b(ctx, tc, out, fmt, nmt):
    from concourse import mybir

    nc = tc.nc
    f32 = mybir.dt.float32
    bf16 = mybir.dt.bfloat16
    AF = mybir.ActivationFunctionType

    sbuf = ctx.enter_context(tc.tile_pool(name="work", bufs=1))
    lpsum = ctx.enter_context(tc.tile_pool(name="lpsum", bufs=1, space="PSUM"))

    # Warm the ACT exp table first thing on the scalar engine, before its DMA.
    warm_in = sbuf.tile([1, 1], f32)
    nc.vector.memset(warm_in[:], 0.0)
    warm_out = sbuf.tile([1, 1], f32)
    nc.scalar.activation(warm_out[:], warm_in[:], AF.Exp)

    NF = KD * BL + KD * B
    nf = sbuf.tile([128, NF], bf16)
    # chunk 0 = nmt + fmt k=0 (gates the first matmul), then one chunk per k.
    edges = [0, KD * BL + B, KD * BL + 2 * B, KD * BL + 3 * B, NF]
    qs = [nc.sync, nc.scalar, nc.sync, nc.scalar]
    for ci in range(4):
        qs[ci].dma_start(nf[:, edges[ci]:edges[ci + 1]], fmt[:, edges[ci]:edges[ci + 1]])
    nt = nf[:, :KD * BL]
    ff = nf[:, KD * BL:]

    lg = lpsum.tile([BL, B], f32)
    for k in range(KD):
        nc.tensor.matmul(
            lg[:], nt[:, k * BL:(k + 1) * BL], ff[:, k * B:(k + 1) * B],
            start=(k == 0), stop=(k == KD - 1),
        )

    # rowsum[b] = sum_c exp(logits[b, c]) via ACT fused row-accumulate.
    # Padded to 128 f32/row: sub-512B HBM writes pay a RMW completion penalty.
    rs = sbuf.tile([BL, 128], f32)
    nc.vector.memset(rs[:], 0.0)
    e_sb = sbuf.tile([BL, B], f32)
    nc.scalar.activation(e_sb[:], lg[:], AF.Exp, accum_out=rs[:, 0:1])
    nc.sync.dma_start(out, rs[:])


def _build_a():
    import concourse.tile as tile
    from concourse import bacc, mybir

    bf16 = mybir.dt.bfloat16
    fp8 = mybir.dt.float8e4
    nc = bacc.Bacc("TRN2", target_bir_lowering=False, debug=False, num_devices=M)
    data = nc.dram_tensor("data", [128, 2 * PCOLS], fp8, kind="ExternalInput").ap()
    sel4 = nc.dram_tensor("sel4", [128, BL], fp8, kind="ExternalInput").ap()
    means = nc.dram_tensor("means", [BL, 2 * D], bf16, kind="ExternalOutput").ap()
    with tile.TileContext(nc) as tc:
        with ExitStack() as ctx:
            _emit_a(ctx, tc, means, data, sel4)
    nc.compile()
    return nc


def _build_b():
    import concourse.tile as tile
    from concourse import bacc, mybir

    f32 = mybir.dt.float32
    bf16 = mybir.dt.bfloat16
    nc = bacc.Bacc("TRN2", target_bir_lowering=False, debug=False, num_devices=M)
    fmt = nc.dram_tensor("fmt", [128, KD * BL + KD * B], bf16, kind="ExternalInput").ap()
    nmt = None
    out = nc.dram_tensor("out", [BL, 128], f32, kind="ExternalOutput").ap()
    with tile.TileContext(nc) as tc:
        with ExitStack() as ctx:
            _emit_b(ctx, tc, out, fmt, nmt)
    nc.compile()
    return nc


def get_nc_a():
    if "a" not in _CACHE:
        _CACHE["a"] = _build_a()
    return _CACHE["a"]


def get_nc_b():
    if "b" not in _CACHE:
        _CACHE["b"] = _build_b()
    return _CACHE["b"]


def _pack_a(x):
    # [32, 32, 512] -> [p = 4m + n//8, j = n%8, d] -> [128, 4096], j-major lines
    fp8 = ml_dtypes.float8_e4m3fn
    return np.asarray(x, dtype=np.float32).reshape(128, PCOLS).astype(fp8)


def build_in_maps_a(face_j, ner_j):
    bf16 = ml_dtypes.bfloat16
    sel4 = np.zeros((128, BL), ml_dtypes.float8_e4m3fn)
    sel4[np.arange(128), np.arange(128) // 4] = np.float32(1.0 / N1)
    maps = []
    for c in range(M):
        sl = slice(c * BL, (c + 1) * BL)
        data = np.concatenate([_pack_a(ner_j[sl]), _pack_a(face_j[sl])], axis=1)
        maps.append({"data": np.ascontiguousarray(data), "sel4": sel4})
    return maps


def _t_km(x):
    # [rows, 512] -> [d' = 128, k*rows + r] (k-major columns), contiguous
    rows = x.shape[0]
    return np.ascontiguousarray(
        x.reshape(rows, KD, 128).transpose(2, 1, 0).reshape(128, KD * rows)
    )


def build_in_maps_b(results_a):
    bf16 = ml_dtypes.bfloat16
    nm = [r["means"][:, :D].astype(np.float32) for r in results_a]
    fm = [r["means"][:, D:].astype(np.float32) for r in results_a]
    fmt = _t_km(np.concatenate(fm, axis=0)).astype(bf16)
    return [
        {"fmt": np.ascontiguousarray(
            np.concatenate([_t_km(nm[c]).astype(bf16), fmt], axis=1))}
        for c in range(M)
    ]


def host_diag(results_a):
    # diag logit for core c's rows: <nm_c[i], fm_c[i]> in f32
    return np.concatenate(
        [
            (
                r["means"][:, :D].astype(np.float32)
                * r["means"][:, D:].astype(np.float32)
            ).sum(axis=1)
            for r in results_a
        ]
    )


def combine(results_a, results_b):
    diag = host_diag(results_a)
    rsum = np.concatenate([r["out"][:, 0] for r in results_b])
    return np.asarray(-np.mean(diag - np.log(rsum)), dtype=np.float32)


def _ensure_ntff_hook():
    """The agent image's antenv lacks axon_hooks; synthesize it and register the
    ctypes NTFF hook from trn_agent_boot so trace=True profiling works."""
    import sys
    import types

    try:
        from antenv.axon_hooks import get_axon_ntff_profile_hook  # noqa: F401

        return
    except ImportError:
        pass
    import antenv
    from trn_agent_boot.trn_boot import _ntff_profile_via_ctypes

    mod = types.ModuleType("antenv.axon_hooks")
    state = {"hook": None}
    mod.set_axon_ntff_profile_hook = lambda h: state.__setitem__("hook", h)
    mod.get_axon_ntff_profile_hook = lambda: state["hook"]
    sys.modules["antenv.axon_hooks"] = mod
    antenv.axon_hooks = mod
    mod.set_axon_ntff_profile_hook(_ntff_profile_via_ctypes("/opt/axon/libaxon_pjrt.so"))


def run_stage(nc, in_maps, trace=False, **kw):
    from concourse import bass_utils

    if trace:
        _ensure_ntff_hook()
    return bass_utils.run_bass_kernel_spmd(
        nc, in_maps, core_ids=list(range(M)), trace=trace, **kw
    )


def kernel(face_j, ner_j):
    res_a = run_stage(get_nc_a(), build_in_maps_a(face_j, ner_j))
    res_b = run_stage(get_nc_b(), build_in_maps_b(res_a.results))
    return combine(res_a.results, res_b.results)


# revision 7
# speedup vs baseline: 1.2076x; 1.2076x over previous
"""Trainium2 Bass kernel for nn_BatchSoftmaxNomax (batch contrastive softmax loss).

Math: scores[b,c,n,f] = <ner[b,n,:], face[c,f,:]>, logits = scores.mean((n,f)),
loss = -mean_b log_softmax(logits)[b,b].
Since the span-means are linear, logits[b,c] = <mean_n ner[b], mean_f face[c]>,
so the O(B^2*N^2*D) einsum collapses to two mean-reductions + a [B,D]x[D,B] matmul.

Sharding (8 cores, batch-sharded), two launches with a host-side gather between
them (a device AllGather costs ~55us of cross-rank launch-skew wait through this
runtime — measured — so two independent launches win).

Launch A (per core, 32 batch rows): host packs both input slices as ONE fp8
tensor [128, 8192] laid out d-major/span-minor (p = 4m + n//8, line = [d, j]),
so each of the 4 streaming DMAs is 2KB-contiguous per partition. fp8 halves the
stream vs bf16; the span-mean averages quantization noise (~1e-3 on the loss).
DVE tensor_reduce(axis=X) sums the 8 spans per partition in one op per tile,
a bf16 cast + one [128,32]x[128,512] PE matmul per tensor finishes the
32-row mean. Output: packed [32, 1024] bf16 means (nm | fm).

Host: gathers/transposes the means into fmt [128, 1024] (k-major) and per-core
nmt [128, 128], computes the 256 diagonal dot products in f32.

Launch B (per core): 3 contiguous DMAs, 4 accumulating bf16 matmuls -> [32, 256]
logits in PSUM; ACT exp with fused row-sum accumulate -> sum_c exp(logits).
Host: loss = -mean(diag - log(rowsum)).
"""

import ml_dtypes
import numpy as np
from contextlib import ExitStack

B = 256      # global batch
N1 = 32      # ner spans
N2 = 32      # face spans
D = 512      # embed dim
M = 8        # cores
BL = B // M  # local batch rows per core (32)
KD = D // 128  # d-chunks (4)
PJ = 8       # spans folded into each partition line
PCOLS = D * PJ          # 4096 fp8 bytes per partition per tensor
NTILE = PCOLS // 2      # 2048 — half-tensor DMA tile width

_CACHE = {}


def _emit_a(ctx, tc, means_out, data, sel4):
    from concourse import mybir

    nc = tc.nc
    f32 = mybir.dt.float32
    bf16 = mybir.dt.bfloat16
    fp8 = mybir.dt.float8e4

    consts = ctx.enter_context(tc.tile_pool(name="consts", bufs=1))
    chunks = ctx.enter_context(tc.tile_pool(name="chunks", bufs=1))
    work = ctx.enter_context(tc.tile_pool(name="work", bufs=1))
    mpsum = ctx.enter_context(tc.tile_pool(name="mpsum", bufs=2, space="PSUM"))

    # sel rides the gpsimd SWDGE so both HWDGE rings start on payload
    # immediately. 4 streaming tiles of 256KB (2KB contiguous per partition
    # line; j-major [j, d] lines), same-tensor halves on different rings.
    sel_sb = consts.tile([128, 2 * BL], fp8)
    nc.gpsimd.dma_start(sel_sb[:], sel4)
    tiles = []
    order = [(nc.sync, 0), (nc.scalar, 1), (nc.sync, 2), (nc.scalar, 3)]
    for q, t in order:
        tl = chunks.tile([128, NTILE], fp8, tag=f"t{t}", name=f"t{t}")
        q.dma_start(tl[:], data[:, t * NTILE:(t + 1) * NTILE])
        tiles.append((t, tl))
    tiles.sort()

    # Span-mean on PE in fp8 DoubleRow perf mode: each matmul consumes TWO
    # j-slices ([128, 2, 512] moving, sel duplicated across the k-pair), so
    # a tensor's 8-span sum is 4 accumulating matmuls at 2x throughput.
    sel_k = sel_sb[:].rearrange("p (k m) -> p k m", k=2)
    ps = [
        mpsum.tile([BL, D], f32, tag=f"ps{i}", name=f"ps{i}")
        for i in range(2)
    ]
    means = work.tile([BL, 2 * D], bf16, tag="means")
    copy_eng = [nc.vector.tensor_copy, nc.scalar.copy]
    from concourse.mybir import MatmulPerfMode
    for i in range(2):
        for half in (0, 1):
            view = tiles[2 * i + half][1][:].rearrange("p (j d) -> p j d", j=PJ // 2)
            for jp in range(PJ // 4):
                nc.tensor.matmul(
                    ps[i][:], sel_k, view[:, 2 * jp:2 * jp + 2, :],
                    start=(half == 0 and jp == 0),
                    stop=(half == 1 and jp == PJ // 4 - 1),
                    perf_mode=MatmulPerfMode.DoubleRow,
                )
        copy_eng[i](means[:, i * D:(i + 1) * D], ps[i][:])
        nc.sync.dma_start(
            means_out[:, i * D:(i + 1) * D], means[:, i * D:(i + 1) * D]
        )


def _emit_b(ctx, tc, out, fmt, nmt):
    from concourse import mybir

    nc = tc.nc
    f32 = mybir.dt.float32
    bf16 = mybir.dt.bfloat16
    AF = mybir.ActivationFunctionType

    sbuf = ctx.enter_context(tc.tile_pool(name="work", bufs=1))
    lpsum = ctx.enter_context(tc.tile_pool(name="lpsum", bufs=1, space="PSUM"))

    # Warm the ACT exp table first thing on the scalar engine, before its DMA.
    warm_in = sbuf.tile([1, 1], f32)
    nc.vector.memset(warm_in[:], 0.0)
    warm_out = sbuf.tile([1, 1], f32)
    nc.scalar.activation(warm_out[:], warm_in[:], AF.Exp)

    NF = KD * BL + KD * B
    nf = sbuf.tile([128, NF], bf16)
    # chunk 0 = nmt + fmt k=0 (gates the first matmul), then one chunk per k.
    edges = [0, KD * BL + B, KD * BL + 2 * B, KD * BL + 3 * B, NF]
    qs = [nc.sync, nc.scalar, nc.sync, nc.scalar]
    for ci in range(4):
        qs[ci].dma_start(nf[:, edges[ci]:edges[ci + 1]], fmt[:, edges[ci]:edges[ci + 1]])
    nt = nf[:, :KD * BL]
    ff = nf[:, KD * BL:]

    lg = lpsum.tile([BL, B], f32)
    for k in range(KD):
        nc.tensor.matmul(
            lg[:], nt[:, k * BL:(k + 1) * BL], ff[:, k * B:(k + 1) * B],
            start=(k == 0), stop=(k == KD - 1),
        )

    # rowsum[b] = sum_c exp(logits[b, c]) via ACT fused row-accumulate.
    # Padded to 128 f32/row: sub-512B HBM writes pay a RMW completion penalty.
    rs = sbuf.tile([BL, 128], f32)
    nc.vector.memset(rs[:], 0.0)
    e_sb = sbuf.tile([BL, B], f32)
    nc.scalar.activation(e_sb[:], lg[:], AF.Exp, accum_out=rs[:, 0:1])
    nc.sync.dma_start(out, rs[:])


def _build_a():
    import concourse.tile as tile
    from concourse import bacc, mybir

    bf16 = mybir.dt.bfloat16
    fp8 = mybir.dt.float8e4
    nc = bacc.Bacc("TRN2", target_bir_lowering=False, debug=False, num_devices=M)
    data = nc.dram_tensor("data", [128, 2 * PCOLS], fp8, kind="ExternalInput").ap()
    sel4 = nc.dram_tensor("sel4", [128, 2 * BL], fp8, kind="ExternalInput").ap()
    means = nc.dram_tensor("means", [BL, 2 * D], bf16, kind="ExternalOutput").ap()
    with tile.TileContext(nc) as tc:
        with ExitStack() as ctx:
            _emit_a(ctx, tc, means, data, sel4)
    nc.compile()
    return nc


def _build_b():
    import concourse.tile as tile
    from concourse import bacc, mybir

    f32 = mybir.dt.float32
    bf16 = mybir.dt.bfloat16
    nc = bacc.Bacc("TRN2", target_bir_lowering=False, debug=False, num_devices=M)
    fmt = nc.dram_tensor("fmt", [128, KD * BL + KD * B], bf16, kind="ExternalInput").ap()
    nmt = None
    out = nc.dram_tensor("out", [BL, 128], f32, kind="ExternalOutput").ap()
    with tile.TileContext(nc) as tc:
        with ExitStack() as ctx:
            _emit_b(ctx, tc, out, fmt, nmt)
    nc.compile()
    return nc


def get_nc_a():
    if "a" not in _CACHE:
        _CACHE["a"] = _build_a()
    return _CACHE["a"]


def get_nc_b():
    if "b" not in _CACHE:
        _CACHE["b"] = _build_b()
    return _CACHE["b"]


def _pack_a(x):
    # [32, 32, 512] -> [p = 4m + n//8, j = n%8, d] -> [128, 4096], j-major lines
    fp8 = ml_dtypes.float8_e4m3fn
    return np.asarray(x, dtype=np.float32).reshape(128, PCOLS).astype(fp8)


def build_in_maps_a(face_j, ner_j):
    bf16 = ml_dtypes.bfloat16
    sel1 = np.zeros((128, BL), ml_dtypes.float8_e4m3fn)
    sel1[np.arange(128), np.arange(128) // 4] = np.float32(1.0 / N1)
    sel4 = np.concatenate([sel1, sel1], axis=1)
    maps = []
    for c in range(M):
        sl = slice(c * BL, (c + 1) * BL)
        data = np.concatenate([_pack_a(ner_j[sl]), _pack_a(face_j[sl])], axis=1)
        maps.append({"data": np.ascontiguousarray(data), "sel4": sel4})
    return maps


def _t_km(x):
    # [rows, 512] -> [d' = 128, k*rows + r] (k-major columns), contiguous
    rows = x.shape[0]
    return np.ascontiguousarray(
        x.reshape(rows, KD, 128).transpose(2, 1, 0).reshape(128, KD * rows)
    )


def build_in_maps_b(results_a):
    bf16 = ml_dtypes.bfloat16
    nm = [r["means"][:, :D].astype(np.float32) for r in results_a]
    fm = [r["means"][:, D:].astype(np.float32) for r in results_a]
    fmt = _t_km(np.concatenate(fm, axis=0)).astype(bf16)
    return [
        {"fmt": np.ascontiguousarray(
            np.concatenate([_t_km(nm[c]).astype(bf16), fmt], axis=1))}
        for c in range(M)
    ]


def host_diag(results_a):
    # diag logit for core c's rows: <nm_c[i], fm_c[i]> in f32
    return np.concatenate(
        [
            (
                r["means"][:, :D].astype(np.float32)
                * r["means"][:, D:].astype(np.float32)
            ).sum(axis=1)
            for r in results_a
        ]
    )


def combine(results_a, results_b):
    diag = host_diag(results_a)
    rsum = np.concatenate([r["out"][:, 0] for r in results_b])
    return np.asarray(-np.mean(diag - np.log(rsum)), dtype=np.float32)


def _ensure_ntff_hook():
    """The agent image's antenv lacks axon_hooks; synthesize it and register the
    ctypes NTFF hook from trn_agent_boot so trace=True profiling works."""
    import sys
    import types

    try:
        from antenv.axon_hooks import get_axon_ntff_profile_hook  # noqa: F401

        return
    except ImportError:
        pass
    import antenv
    from trn_agent_boot.trn_boot import _ntff_profile_via_ctypes

    mod = types.ModuleType("antenv.axon_hooks")
    state = {"hook": None}
    mod.set_axon_ntff_profile_hook = lambda h: state.__setitem__("hook", h)
    mod.get_axon_ntff_profile_hook = lambda: state["hook"]
    sys.modules["antenv.axon_hooks"] = mod
    antenv.axon_hooks = mod
    mod.set_axon_ntff_profile_hook(_ntff_profile_via_ctypes("/opt/axon/libaxon_pjrt.so"))


def run_stage(nc, in_maps, trace=False, **kw):
    from concourse import bass_utils

    if trace:
        _ensure_ntff_hook()
    return bass_utils.run_bass_kernel_spmd(
        nc, in_maps, core_ids=list(range(M)), trace=trace, **kw
    )


def kernel(face_j, ner_j):
    res_a = run_stage(get_nc_a(), build_in_maps_a(face_j, ner_j))
    res_b = run_stage(get_nc_b(), build_in_maps_b(res_a.results))
    return combine(res_a.results, res_b.results)
